# revision 30
# baseline (speedup 1.0000x reference)
"""GNN node-update layer (segment-softmax message passing) on 8 TRN2 cores.

Math notes (validated vs reference in fp64: rel L2 ~1.5e-6):
  - scores_mean = h_dst @ mean(attn_W)[:128] + ef @ mean(attn_W)[128:] + mean(b).
    The h_dst term is constant within each dst segment, so it cancels in the
    segment softmax -> no node gather needed.  q_e = ef_e @ w_q.
  - q in [-0.9, 0.9] so exp(q) needs no max subtraction; the reference's
    max-shift only changes the EPS term by O(1e-5) relative.
  - softmax-weighted sum of edge values pushed through the linear projection:
      msg_n = (G_n @ ev_W)/(S_n+eps) + ev_b * S_n/(S_n+eps),
      G_n = sum_e exp(q_e) ef_e  (64-dim),  S_n = sum_e exp(q_e).
  - out = nf + relu(msg@W1+b1)@W2 + b2   (b2 folded into the nf stream).

Sharding: nodes partitioned into 8 contiguous ranges of 12500; each core gets
the edges whose dst lands in its range (no collectives).  Host sorts edges by
dst, groups them by 128-node tile, pads each group to 8 chunks of 128 edge
slots (max real count is ~900).  Pad slots carry dst sentinel 128 so their
one-hot column is all-zero and they contribute nothing.

Edge features ride in two fp16 halves (hi + residual lo) so the scatter
matmul runs at 1 cycle/row with ~fp32 accuracy: one 130-wide rhs
[ef_hi | 1 | ef_lo | 0] per chunk, hi/lo partial sums added in the epilogue.

Device per 128-edge chunk:
  q-mm   : lhsT = efT(65x128) fp16, rhs = wq(65x1)      -> q column (128x1)
  exp    : ACT on the 8 q columns of a group -> exp_all slice (128x8)
  one-hot: DVE ohx = (iota == dst) * exp  (two-op tensor_scalar, fp16 out)
  scatter: lhsT = ohx, rhs = combo(128x130) fp16, psum-accumulated over the
           group -> [G_hi | S | G_lo | 0] (128n x 130)
Group epilogue: g65 = hi+lo; r = 1/(S+eps); gh = g65*r; PE-transpose ->
  ghT(65x128).  Node batch (4 groups): msgT = evW65^T @ ghT;
  t1 = (W1^T msgT + b1) max 0 (DVE); t2 = W2^T t1; out = t2 + nfT -> DMA.

Hardware constraint honored throughout: matmul / tensor-scalar encodings fit
only ONE sync-wait, so every hot instruction is arranged to have exactly one
cross-engine dependency.  DMA-completion waits are absorbed by throwaway PE
load_weights / transpose reads ("absorb" ops); exp_all and the deep oh pool
avoid write-after-read waits entirely.
"""

import sys

sys.path.insert(0, "/opt/trn_rl_repo")

import numpy as np

import concourse.bass as bass
import concourse.bacc as bacc
import concourse.mybir as mybir
from concourse.tile import TileContext, add_dep_helper
from concourse.bass_utils import run_bass_kernel_spmd

P = 128
N_CORES = 8
N_NODES = 100000
NPC = 12500                # nodes per core
NTILE = 98                 # 128-node tiles per core
NODE_PAD = NTILE * P       # 12544
CPG = 8                    # chunks (of 128 edge slots) per node tile
NCH = NTILE * CPG          # 784 chunks per core
E_PAD = NCH * P            # 100352 edge slots per core
GB = 4                     # node tiles per MLP batch (N = 512)
EPS = 1e-6

F32 = mybir.dt.float32
F32R = mybir.dt.float32r
F16 = mybir.dt.float16


def _build_nc():
    nc = bacc.Bacc()
    eft_d = nc.dram_tensor("eft", (65, E_PAD), F16, kind="ExternalInput")
    efp_d = nc.dram_tensor("efp", (P, NCH * 130), F16, kind="ExternalInput")
    dst_d = nc.dram_tensor("dstc", (P, NCH), F32, kind="ExternalInput")
    nft_d = nc.dram_tensor("nft", (P, NODE_PAD), F32, kind="ExternalInput")
    wq_d = nc.dram_tensor("wq", (65, 1), F16, kind="ExternalInput")
    evw_d = nc.dram_tensor("evw", (65, P), F32R, kind="ExternalInput")
    w1_d = nc.dram_tensor("w1", (P, P), F32R, kind="ExternalInput")
    w2_d = nc.dram_tensor("w2", (P, P), F32R, kind="ExternalInput")
    b1_d = nc.dram_tensor("b1", (P, 1), F32, kind="ExternalInput")
    iota_d = nc.dram_tensor("iota", (P, P), F16, kind="ExternalInput")
    ident_d = nc.dram_tensor("ident", (P, P), F32, kind="ExternalInput")
    out_d = nc.dram_tensor("outt", (P, NODE_PAD), F32, kind="ExternalOutput")

    with TileContext(nc) as tc:
        with (
            tc.tile_pool(name="const", bufs=1) as cp,
            tc.tile_pool(name="sbuf", bufs=3) as sb,
            tc.tile_pool(name="quad", bufs=4) as sb4,
            tc.tile_pool(name="stream", bufs=16) as sb16,
            tc.tile_pool(name="ohp", bufs=12) as ohp,
            tc.tile_pool(name="pq", bufs=2, space="PSUM") as pq_pool,
            tc.tile_pool(name="pg", bufs=2, space="PSUM") as pg_pool,
            tc.tile_pool(name="pt", bufs=1, space="PSUM") as pt_pool,
            tc.tile_pool(name="pbig", bufs=3, space="PSUM") as pbig_pool,
        ):
            def dma(out, in_):
                return nc.sync.dma_start(out=out, in_=in_)

            wq_t = cp.tile([65, 1], F16)
            dma(out=wq_t[:], in_=wq_d[:])
            evw_t = cp.tile([65, P], F32R)
            dma(out=evw_t[:], in_=evw_d[:])
            w1_t = cp.tile([P, P], F32R)
            dma(out=w1_t[:], in_=w1_d[:])
            w2_t = cp.tile([P, P], F32R)
            dma(out=w2_t[:], in_=w2_d[:])
            b1_t = cp.tile([P, 1], F32)
            dma(out=b1_t[:], in_=b1_d[:])
            iota_t = cp.tile([P, P], F16)
            dma(out=iota_t[:], in_=iota_d[:])
            ident_t = cp.tile([P, P], F32)
            dma(out=ident_t[:], in_=ident_d[:])
            dst_t = cp.tile([P, NCH], F32)
            dma(out=dst_t[:], in_=dst_d[:])
            # exp_all: one long-lived tile -> ACT never sees a WAR wait.
            exp_all = cp.tile([P, NCH], F32)
            # fresh-column scratch: each col written exactly once, so the
            # writing instruction carries only its single genuine wait.
            sdend = cp.tile([P, NTILE], F32)    # S + eps
            recd = cp.tile([P, NTILE], F32)     # 1/(S+eps)
            ob_big = cp.tile([P, NODE_PAD], F32)  # full output staging

            batches = [
                list(range(s, min(s + GB, NTILE))) for s in range(0, NTILE, GB)
            ]
            for groups in batches:
                gbw = len(groups) * P
                g0 = groups[0]
                ecols = len(groups) * CPG * P
                ccols = len(groups) * CPG * 130
                eft_b = sb4.tile([65, GB * CPG * P], F16, tag="eftb")
                nc.scalar.dma_start(out=eft_b[:, :ecols],
                    in_=eft_d[:, g0 * CPG * P : g0 * CPG * P + ecols])
                combo_b = sb4.tile([P, GB * CPG * 130], F16, tag="combob")
                dma(out=combo_b[:, :ccols],
                    in_=efp_d[:, g0 * CPG * 130 : g0 * CPG * 130 + ccols])
                ghT_b = sb.tile([65, GB * P], F32R, tag="ghT")
                for j, g in enumerate(groups):
                    eft_sl = eft_b[:, j * CPG * P : (j + 1) * CPG * P]
                    combo = combo_b[:, j * CPG * 130 : (j + 1) * CPG * 130]
                    pq = pq_pool.tile([P, CPG], F32, tag="q")
                    for k in range(CPG):
                        nc.tensor.matmul(
                            out=pq[:, k : k + 1],
                            lhsT=eft_sl[:, k * P : (k + 1) * P],
                            rhs=wq_t[:],
                            start=True,
                            stop=True,
                        )
                    nc.scalar.activation(
                        out=exp_all[:, g * CPG : (g + 1) * CPG],
                        in_=pq[:],
                        func=mybir.ActivationFunctionType.Exp,
                    )
                    pg = pg_pool.tile([P, 130], F32, tag="G")
                    for k in range(CPG):
                        c = g * CPG + k
                        oh = ohp.tile([P, P], F16, tag="oh")
                        eng = nc.vector if (k % 2 == 0) else nc.gpsimd
                        eng.tensor_scalar(
                            oh[:],
                            iota_t[:],
                            dst_t[:, c : c + 1],
                            exp_all[:, c : c + 1],
                            mybir.AluOpType.is_equal,
                            mybir.AluOpType.mult,
                        )
                        nc.tensor.matmul(
                            out=pg[:],
                            lhsT=oh[:],
                            rhs=combo[:, k * 130 : (k + 1) * 130],
                            start=(k == 0),
                            stop=(k == CPG - 1),
                        )
                    # epilogue: r = 1/(S+eps); gh_{hi,lo} = G_{hi,lo}*r; the
                    # PE transpose pair adds hi+lo via PSUM accumulation.
                    nc.vector.tensor_scalar_add(
                        sdend[:, g : g + 1], pg[:, 64:65], EPS)
                    nc.vector.reciprocal(recd[:, g : g + 1], sdend[:, g : g + 1])
                    gh_full = sb.tile([P, 130], F32, tag="gh")
                    nc.vector.tensor_scalar_mul(gh_full[:], pg[:],
                                                recd[:, g : g + 1])
                    gh_hi = gh_full[:, 0:65]
                    gh_lo = gh_full[:, 65:130]
                    ptr = pt_pool.tile([P, P], F32, tag="T")
                    nc.tensor.matmul(
                        out=ptr[0:65, :], lhsT=gh_hi, rhs=ident_t[:],
                        is_transpose=True, start=True, stop=False,
                    )
                    nc.tensor.matmul(
                        out=ptr[0:65, :], lhsT=gh_lo, rhs=ident_t[:],
                        is_transpose=True, start=False, stop=True,
                    )
                    nc.any.tensor_copy(
                        out=ghT_b[:, j * P : (j + 1) * P], in_=ptr[0:65, :]
                    )
                # node-side MLP batch
                col0 = groups[0] * P
                pmsg = pbig_pool.tile([P, GB * P], F32, tag="big")
                nc.tensor.matmul(
                    out=pmsg[:, :gbw],
                    lhsT=evw_t[:],
                    rhs=ghT_b[:, :gbw],
                    start=True,
                    stop=True,
                )
                msgT = sb.tile([P, GB * P], F32R, tag="msgT")
                nc.any.tensor_copy(out=msgT[:, :gbw], in_=pmsg[:, :gbw])
                pt1 = pbig_pool.tile([P, GB * P], F32, tag="big")
                nc.tensor.matmul(
                    out=pt1[:, :gbw],
                    lhsT=w1_t[:],
                    rhs=msgT[:, :gbw],
                    start=True,
                    stop=True,
                )
                t1 = sb.tile([P, GB * P], F32R, tag="t1")
                nc.vector.tensor_scalar(
                    t1[:, :gbw],
                    pt1[:, :gbw],
                    b1_t[:, 0:1],
                    0.0,
                    mybir.AluOpType.add,
                    mybir.AluOpType.max,
                )
                nf_sl = sb4.tile([P, GB * P], F32, tag="nf")
                nc.gpsimd.dma_start(
                    out=nf_sl[:, :gbw], in_=nft_d[:, col0 : col0 + gbw])
                pt2 = pbig_pool.tile([P, GB * P], F32, tag="big")
                nc.tensor.matmul(
                    out=pt2[:, :gbw],
                    lhsT=w2_t[:],
                    rhs=t1[:, :gbw],
                    start=True,
                    stop=False,
                )
                # residual add on PE: identity-matmul accumulates nfT into
                # the same PSUM tile; its only wait is the nf DMA.
                nc.tensor.matmul(
                    out=pt2[:, :gbw],
                    lhsT=ident_t[:],
                    rhs=nf_sl[:, :gbw],
                    start=False,
                    stop=True,
                )
                nc.any.tensor_copy(
                    out=ob_big[:, col0 : col0 + gbw], in_=pt2[:, :gbw])
            # single output DMA: one DVE wait, no slot churn; off-chain
            # (it is last, lane phase no longer matters)
            nc.gpsimd.dma_start(out=out_d[:], in_=ob_big[:])
    nc.finalize()
    return nc


def _prep(inputs):
    nf = np.asarray(inputs["node_features"], np.float32)
    ef = np.asarray(inputs["edge_features"], np.float32)
    dst = np.asarray(inputs["edge_index"])[1].astype(np.int64)
    attn_w = np.asarray(inputs["attn_W"], np.float64).mean(axis=1)
    w_q = attn_w[128:].astype(np.float32)

    order = np.argsort(dst, kind="stable")
    ef_s = ef[order]
    dst_s = dst[order]
    bounds = np.searchsorted(dst_s, np.arange(0, N_NODES + 1, NPC))

    evw65 = np.concatenate(
        [np.asarray(inputs["ev_W"], np.float32),
         np.asarray(inputs["ev_b"], np.float32)[None, :]], axis=0)
    wq65 = np.concatenate([w_q, np.zeros(1, np.float32)])[:, None]
    b2 = np.asarray(inputs["out2_b"], np.float32)

    common = {
        "wq": wq65.astype(np.float16),
        "evw": np.ascontiguousarray(evw65),
        "w1": np.ascontiguousarray(np.asarray(inputs["out1_W"], np.float32)),
        "w2": np.ascontiguousarray(np.asarray(inputs["out2_W"], np.float32)),
        "b1": np.ascontiguousarray(
            np.asarray(inputs["out1_b"], np.float32)[:, None]),
        "iota": np.tile(np.arange(P, dtype=np.float16), (P, 1)),
        "ident": np.eye(P, dtype=np.float32),
    }

    in_maps = []
    for core in range(N_CORES):
        lo, hi = bounds[core], bounds[core + 1]
        efk = ef_s[lo:hi]
        dstk = dst_s[lo:hi] - core * NPC
        tile_id = dstk >> 7
        counts = np.bincount(tile_id, minlength=NTILE)
        assert counts.max() <= CPG * P, f"tile overflow: {counts.max()}"
        cum = np.zeros(NTILE, np.int64)
        np.cumsum(counts[:-1], out=cum[1:])
        slots = tile_id * (CPG * P) + (np.arange(len(dstk)) - cum[tile_id])

        ef_pad = np.zeros((E_PAD, 65), np.float32)
        ef_pad[slots, :64] = efk
        ef_pad[:, 64] = 1.0
        hi16 = ef_pad.astype(np.float16)
        lo16 = (ef_pad - hi16.astype(np.float32)).astype(np.float16)
        block = np.concatenate([hi16, lo16], axis=1)  # (E_PAD, 130)
        efp = np.ascontiguousarray(
            block.reshape(NCH, P, 130).transpose(1, 0, 2).reshape(P, NCH * 130))

        dl = np.full(E_PAD, P, np.float32)
        dl[slots] = (dstk & 127).astype(np.float32)
        dstc = np.ascontiguousarray(dl.reshape(NCH, P).T)

        eft = np.empty((65, E_PAD), np.float16)
        eft[:64] = ef_pad[:, :64].T
        eft[64] = 1.0

        nfk = np.zeros((NODE_PAD, P), np.float32)
        nfk[:NPC] = nf[core * NPC : (core + 1) * NPC]
        nft = np.ascontiguousarray(nfk.T + b2[:, None])

        in_maps.append(
            dict(common, eft=np.ascontiguousarray(eft), efp=efp,
                 dstc=dstc, nft=nft))
    return in_maps


_CACHED_NC = None


def kernel(**inputs) -> np.ndarray:
    global _CACHED_NC
    in_maps = _prep(inputs)
    if _CACHED_NC is None:
        _CACHED_NC = _build_nc()
    res = run_bass_kernel_spmd(_CACHED_NC, in_maps, core_ids=list(range(N_CORES)))
    return assemble(res.results)


def assemble(results):
    out = np.empty((N_NODES, P), np.float32)
    for core in range(N_CORES):
        out[core * NPC : (core + 1) * NPC] = results[core]["outt"][:, :NPC].T
    return out


if __name__ == "__main__":
    rng = np.random.default_rng(0)
    demo = {
        "node_features": rng.standard_normal((N_NODES, P)).astype(np.float32),
        "edge_features": rng.standard_normal((640000, 64)).astype(np.float32),
        "attn_W": (rng.standard_normal((192, 4)) * 0.07).astype(np.float32),
        "attn_b": (rng.standard_normal(4) * 0.07).astype(np.float32),
        "ev_W": (rng.standard_normal((64, P)) * 0.125).astype(np.float32),
        "ev_b": (rng.standard_normal(P) * 0.125).astype(np.float32),
        "out1_W": (rng.standard_normal((P, P)) * 0.09).astype(np.float32),
        "out1_b": (rng.standard_normal(P) * 0.09).astype(np.float32),
        "out2_W": (rng.standard_normal((P, P)) * 0.09).astype(np.float32),
        "out2_b": (rng.standard_normal(P) * 0.09).astype(np.float32),
        "edge_index": rng.integers(0, N_NODES, (2, 640000)).astype(np.int32),
    }
    out = kernel(**demo)
    print("kernel ran:", out.shape, out.dtype, np.abs(out).max())


# revision 34
# speedup vs baseline: 1.0769x; 1.0769x over previous
"""GNN node-update layer (segment-softmax message passing) on 8 TRN2 cores.

Math notes (validated vs reference in fp64: rel L2 ~1.5e-6):
  - scores_mean = h_dst @ mean(attn_W)[:128] + ef @ mean(attn_W)[128:] + mean(b).
    The h_dst term is constant within each dst segment, so it cancels in the
    segment softmax -> no node gather needed.  q_e = ef_e @ w_q.
  - q in [-0.9, 0.9] so exp(q) needs no max subtraction; the reference's
    max-shift only changes the EPS term by O(1e-5) relative.
  - softmax-weighted sum of edge values pushed through the linear projection:
      msg_n = (G_n @ ev_W)/(S_n+eps) + ev_b * S_n/(S_n+eps),
      G_n = sum_e exp(q_e) ef_e  (64-dim),  S_n = sum_e exp(q_e).
  - out = nf + relu(msg@W1+b1)@W2 + b2   (b2 folded into the nf stream).

Sharding: nodes partitioned into 8 contiguous ranges of 12500; each core gets
the edges whose dst lands in its range (no collectives).  Host sorts edges by
dst, groups them by 128-node tile, pads each group to 8 chunks of 128 edge
slots (max real count is ~900).  Pad slots carry dst sentinel 128 so their
one-hot column is all-zero and they contribute nothing.

Edge features ride in two fp16 halves (hi + residual lo) so the scatter
matmul runs at 1 cycle/row with ~fp32 accuracy: one 130-wide rhs
[ef_hi | 1 | ef_lo | 0] per chunk, hi/lo partial sums added in the epilogue.

Device per 128-edge chunk:
  q-mm   : lhsT = efT(65x128) fp16, rhs = wq(65x1)      -> q column (128x1)
  exp    : ACT on the 8 q columns of a group -> exp_all slice (128x8)
  one-hot: DVE ohx = (iota == dst) * exp  (two-op tensor_scalar, fp16 out)
  scatter: lhsT = ohx, rhs = combo(128x130) fp16, psum-accumulated over the
           group -> [G_hi | S | G_lo | 0] (128n x 130)
Group epilogue: g65 = hi+lo; r = 1/(S+eps); gh = g65*r; PE-transpose ->
  ghT(65x128).  Node batch (4 groups): msgT = evW65^T @ ghT;
  t1 = (W1^T msgT + b1) max 0 (DVE); t2 = W2^T t1; out = t2 + nfT -> DMA.

Hardware constraint honored throughout: matmul / tensor-scalar encodings fit
only ONE sync-wait, so every hot instruction is arranged to have exactly one
cross-engine dependency.  DMA-completion waits are absorbed by throwaway PE
load_weights / transpose reads ("absorb" ops); exp_all and the deep oh pool
avoid write-after-read waits entirely.
"""

import sys

sys.path.insert(0, "/opt/trn_rl_repo")

import numpy as np

import concourse.bass as bass
import concourse.bacc as bacc
import concourse.mybir as mybir
from concourse.tile import TileContext, add_dep_helper
from concourse.bass_utils import run_bass_kernel_spmd

P = 128
N_CORES = 8
N_NODES = 100000
NPC = 12500                # nodes per core
NTILE = 98                 # 128-node tiles per core
NODE_PAD = NTILE * P       # 12544
CPG = 8                    # chunks (of 128 edge slots) per node tile
NCH = NTILE * CPG          # 784 chunks per core
E_PAD = NCH * P            # 100352 edge slots per core
GB = 4                     # node tiles per MLP batch (N = 512)
EPS = 1e-6

F32 = mybir.dt.float32
F32R = mybir.dt.float32r
F16 = mybir.dt.float16


def _build_nc():
    nc = bacc.Bacc()
    eft_d = nc.dram_tensor("eft", (65, E_PAD), F16, kind="ExternalInput")
    efp_d = nc.dram_tensor("efp", (P, NCH * 130), F16, kind="ExternalInput")
    dst_d = nc.dram_tensor("dstc", (P, NCH), F32, kind="ExternalInput")
    nft_d = nc.dram_tensor("nft", (P, NODE_PAD), F32, kind="ExternalInput")
    wq_d = nc.dram_tensor("wq", (65, 1), F16, kind="ExternalInput")
    evw_d = nc.dram_tensor("evw", (65, P), F32R, kind="ExternalInput")
    w1_d = nc.dram_tensor("w1", (P, P), F32R, kind="ExternalInput")
    w2_d = nc.dram_tensor("w2", (P, P), F32R, kind="ExternalInput")
    b1_d = nc.dram_tensor("b1", (P, 1), F32, kind="ExternalInput")
    iota_d = nc.dram_tensor("iota", (P, P), F16, kind="ExternalInput")
    ident_d = nc.dram_tensor("ident", (P, P), F32, kind="ExternalInput")
    out_d = nc.dram_tensor("outt", (P, NODE_PAD), F32, kind="ExternalOutput")

    with TileContext(nc) as tc:
        with (
            tc.tile_pool(name="const", bufs=1) as cp,
            tc.tile_pool(name="sbuf", bufs=3) as sb,
            tc.tile_pool(name="quad", bufs=4) as sb4,
            tc.tile_pool(name="stream", bufs=16) as sb16,
            tc.tile_pool(name="ohp", bufs=12) as ohp,
            tc.tile_pool(name="pq", bufs=2, space="PSUM") as pq_pool,
            tc.tile_pool(name="pg", bufs=2, space="PSUM") as pg_pool,
            tc.tile_pool(name="pt", bufs=1, space="PSUM") as pt_pool,
            tc.tile_pool(name="pbig", bufs=3, space="PSUM") as pbig_pool,
        ):
            def dma(out, in_):
                return nc.sync.dma_start(out=out, in_=in_)

            wq_t = cp.tile([65, 1], F16)
            dma(out=wq_t[:], in_=wq_d[:])
            evw_t = cp.tile([65, P], F32R)
            dma(out=evw_t[:], in_=evw_d[:])
            w1_t = cp.tile([P, P], F32R)
            dma(out=w1_t[:], in_=w1_d[:])
            w2_t = cp.tile([P, P], F32R)
            dma(out=w2_t[:], in_=w2_d[:])
            b1_t = cp.tile([P, 1], F32)
            dma(out=b1_t[:], in_=b1_d[:])
            iota_t = cp.tile([P, P], F16)
            dma(out=iota_t[:], in_=iota_d[:])
            ident_t = cp.tile([P, P], F32)
            dma(out=ident_t[:], in_=ident_d[:])
            dst_t = cp.tile([P, NCH], F32)
            dma(out=dst_t[:], in_=dst_d[:])
            # exp_all: one long-lived tile -> ACT never sees a WAR wait.
            exp_all = cp.tile([P, NCH], F32)
            # fresh-column scratch: each col written exactly once, so the
            # writing instruction carries only its single genuine wait.
            sdend = cp.tile([P, NTILE], F32)    # S + eps
            recd = cp.tile([P, NTILE], F32)     # 1/(S+eps)
            ob_big = cp.tile([P, NODE_PAD], F32)  # full output staging

            batches = [
                list(range(s, min(s + GB, NTILE))) for s in range(0, NTILE, GB)
            ]
            for groups in batches:
                gbw = len(groups) * P
                g0 = groups[0]
                ecols = len(groups) * CPG * P
                ccols = len(groups) * CPG * 130
                eft_b = sb4.tile([65, GB * CPG * P], F16, tag="eftb")
                nc.scalar.dma_start(out=eft_b[:, :ecols],
                    in_=eft_d[:, g0 * CPG * P : g0 * CPG * P + ecols])
                combo_b = sb4.tile([P, GB * CPG * 130], F16, tag="combob")
                dma(out=combo_b[:, :ccols],
                    in_=efp_d[:, g0 * CPG * 130 : g0 * CPG * 130 + ccols])
                ghT_b = sb.tile([65, GB * P], F32R, tag="ghT")
                nb = len(groups) * CPG
                pq = pq_pool.tile([P, GB * CPG], F32, tag="q")
                for kk in range(nb):
                    nc.tensor.matmul(
                        out=pq[:, kk : kk + 1],
                        lhsT=eft_b[:, kk * P : (kk + 1) * P],
                        rhs=wq_t[:],
                        start=True,
                        stop=True,
                    )
                nc.scalar.activation(
                    out=exp_all[:, g0 * CPG : g0 * CPG + nb],
                    in_=pq[:, :nb],
                    func=mybir.ActivationFunctionType.Exp,
                )
                for j, g in enumerate(groups):
                    eft_sl = eft_b[:, j * CPG * P : (j + 1) * CPG * P]
                    combo = combo_b[:, j * CPG * 130 : (j + 1) * CPG * 130]
                    pg = pg_pool.tile([P, 130], F32, tag="G")
                    for k in range(CPG):
                        c = g * CPG + k
                        oh = ohp.tile([P, P], F16, tag="oh")
                        eng = nc.vector if (k % 8 < 3) else nc.gpsimd
                        eng.tensor_scalar(
                            oh[:],
                            iota_t[:],
                            dst_t[:, c : c + 1],
                            exp_all[:, c : c + 1],
                            mybir.AluOpType.is_equal,
                            mybir.AluOpType.mult,
                        )
                        nc.tensor.matmul(
                            out=pg[:],
                            lhsT=oh[:],
                            rhs=combo[:, k * 130 : (k + 1) * 130],
                            start=(k == 0),
                            stop=(k == CPG - 1),
                        )
                    # epilogue: r = 1/(S+eps); gh_{hi,lo} = G_{hi,lo}*r; the
                    # PE transpose pair adds hi+lo via PSUM accumulation.
                    nc.vector.tensor_scalar_add(
                        sdend[:, g : g + 1], pg[:, 64:65], EPS)
                    nc.vector.reciprocal(recd[:, g : g + 1], sdend[:, g : g + 1])
                    gh_full = sb.tile([P, 130], F32, tag="gh")
                    nc.vector.tensor_scalar_mul(gh_full[:], pg[:],
                                                recd[:, g : g + 1])
                    gh_hi = gh_full[:, 0:65]
                    gh_lo = gh_full[:, 65:130]
                    ptr = pt_pool.tile([P, P], F32, tag="T")
                    nc.tensor.matmul(
                        out=ptr[0:65, :], lhsT=gh_hi, rhs=ident_t[:],
                        is_transpose=True, start=True, stop=False,
                    )
                    nc.tensor.matmul(
                        out=ptr[0:65, :], lhsT=gh_lo, rhs=ident_t[:],
                        is_transpose=True, start=False, stop=True,
                    )
                    nc.vector.tensor_copy(
                        out=ghT_b[:, j * P : (j + 1) * P], in_=ptr[0:65, :]
                    )
                # node-side MLP batch
                col0 = groups[0] * P
                pmsg = pbig_pool.tile([P, GB * P], F32, tag="big")
                nc.tensor.matmul(
                    out=pmsg[:, :gbw],
                    lhsT=evw_t[:],
                    rhs=ghT_b[:, :gbw],
                    start=True,
                    stop=True,
                )
                msgT = sb.tile([P, GB * P], F32R, tag="msgT")
                nc.scalar.copy(out=msgT[:, :gbw], in_=pmsg[:, :gbw])
                pt1 = pbig_pool.tile([P, GB * P], F32, tag="big")
                nc.tensor.matmul(
                    out=pt1[:, :gbw],
                    lhsT=w1_t[:],
                    rhs=msgT[:, :gbw],
                    start=True,
                    stop=True,
                )
                t1 = sb.tile([P, GB * P], F32R, tag="t1")
                nc.vector.tensor_scalar(
                    t1[:, :gbw],
                    pt1[:, :gbw],
                    b1_t[:, 0:1],
                    0.0,
                    mybir.AluOpType.add,
                    mybir.AluOpType.max,
                )
                nf_sl = sb4.tile([P, GB * P], F32, tag="nf")
                nc.gpsimd.dma_start(
                    out=nf_sl[:, :gbw], in_=nft_d[:, col0 : col0 + gbw])
                pt2 = pbig_pool.tile([P, GB * P], F32, tag="big")
                nc.tensor.matmul(
                    out=pt2[:, :gbw],
                    lhsT=w2_t[:],
                    rhs=t1[:, :gbw],
                    start=True,
                    stop=False,
                )
                # residual add on PE: identity-matmul accumulates nfT into
                # the same PSUM tile; its only wait is the nf DMA.
                nc.tensor.matmul(
                    out=pt2[:, :gbw],
                    lhsT=ident_t[:],
                    rhs=nf_sl[:, :gbw],
                    start=False,
                    stop=True,
                )
                nc.scalar.copy(
                    out=ob_big[:, col0 : col0 + gbw], in_=pt2[:, :gbw])
            # single output DMA: one DVE wait, no slot churn; off-chain
            # (it is last, lane phase no longer matters)
            nc.gpsimd.dma_start(out=out_d[:], in_=ob_big[:])
    nc.finalize()
    return nc


def _prep(inputs):
    nf = np.asarray(inputs["node_features"], np.float32)
    ef = np.asarray(inputs["edge_features"], np.float32)
    dst = np.asarray(inputs["edge_index"])[1].astype(np.int64)
    attn_w = np.asarray(inputs["attn_W"], np.float64).mean(axis=1)
    w_q = attn_w[128:].astype(np.float32)

    order = np.argsort(dst, kind="stable")
    ef_s = ef[order]
    dst_s = dst[order]
    bounds = np.searchsorted(dst_s, np.arange(0, N_NODES + 1, NPC))

    evw65 = np.concatenate(
        [np.asarray(inputs["ev_W"], np.float32),
         np.asarray(inputs["ev_b"], np.float32)[None, :]], axis=0)
    wq65 = np.concatenate([w_q, np.zeros(1, np.float32)])[:, None]
    b2 = np.asarray(inputs["out2_b"], np.float32)

    common = {
        "wq": wq65.astype(np.float16),
        "evw": np.ascontiguousarray(evw65),
        "w1": np.ascontiguousarray(np.asarray(inputs["out1_W"], np.float32)),
        "w2": np.ascontiguousarray(np.asarray(inputs["out2_W"], np.float32)),
        "b1": np.ascontiguousarray(
            np.asarray(inputs["out1_b"], np.float32)[:, None]),
        "iota": np.tile(np.arange(P, dtype=np.float16), (P, 1)),
        "ident": np.eye(P, dtype=np.float32),
    }

    in_maps = []
    for core in range(N_CORES):
        lo, hi = bounds[core], bounds[core + 1]
        efk = ef_s[lo:hi]
        dstk = dst_s[lo:hi] - core * NPC
        tile_id = dstk >> 7
        counts = np.bincount(tile_id, minlength=NTILE)
        assert counts.max() <= CPG * P, f"tile overflow: {counts.max()}"
        cum = np.zeros(NTILE, np.int64)
        np.cumsum(counts[:-1], out=cum[1:])
        slots = tile_id * (CPG * P) + (np.arange(len(dstk)) - cum[tile_id])

        ef_pad = np.zeros((E_PAD, 65), np.float32)
        ef_pad[slots, :64] = efk
        ef_pad[:, 64] = 1.0
        hi16 = ef_pad.astype(np.float16)
        lo16 = (ef_pad - hi16.astype(np.float32)).astype(np.float16)
        block = np.concatenate([hi16, lo16], axis=1)  # (E_PAD, 130)
        efp = np.ascontiguousarray(
            block.reshape(NCH, P, 130).transpose(1, 0, 2).reshape(P, NCH * 130))

        dl = np.full(E_PAD, P, np.float32)
        dl[slots] = (dstk & 127).astype(np.float32)
        dstc = np.ascontiguousarray(dl.reshape(NCH, P).T)

        eft = np.empty((65, E_PAD), np.float16)
        eft[:64] = ef_pad[:, :64].T
        eft[64] = 1.0

        nfk = np.zeros((NODE_PAD, P), np.float32)
        nfk[:NPC] = nf[core * NPC : (core + 1) * NPC]
        nft = np.ascontiguousarray(nfk.T + b2[:, None])

        in_maps.append(
            dict(common, eft=np.ascontiguousarray(eft), efp=efp,
                 dstc=dstc, nft=nft))
    return in_maps


_CACHED_NC = None


def kernel(**inputs) -> np.ndarray:
    global _CACHED_NC
    in_maps = _prep(inputs)
    if _CACHED_NC is None:
        _CACHED_NC = _build_nc()
    res = run_bass_kernel_spmd(_CACHED_NC, in_maps, core_ids=list(range(N_CORES)))
    return assemble(res.results)


def assemble(results):
    out = np.empty((N_NODES, P), np.float32)
    for core in range(N_CORES):
        out[core * NPC : (core + 1) * NPC] = results[core]["outt"][:, :NPC].T
    return out


if __name__ == "__main__":
    rng = np.random.default_rng(0)
    demo = {
        "node_features": rng.standard_normal((N_NODES, P)).astype(np.float32),
        "edge_features": rng.standard_normal((640000, 64)).astype(np.float32),
        "attn_W": (rng.standard_normal((192, 4)) * 0.07).astype(np.float32),
        "attn_b": (rng.standard_normal(4) * 0.07).astype(np.float32),
        "ev_W": (rng.standard_normal((64, P)) * 0.125).astype(np.float32),
        "ev_b": (rng.standard_normal(P) * 0.125).astype(np.float32),
        "out1_W": (rng.standard_normal((P, P)) * 0.09).astype(np.float32),
        "out1_b": (rng.standard_normal(P) * 0.09).astype(np.float32),
        "out2_W": (rng.standard_normal((P, P)) * 0.09).astype(np.float32),
        "out2_b": (rng.standard_normal(P) * 0.09).astype(np.float32),
        "edge_index": rng.integers(0, N_NODES, (2, 640000)).astype(np.int32),
    }
    out = kernel(**demo)
    print("kernel ran:", out.shape, out.dtype, np.abs(out).max())


# revision 40
# speedup vs baseline: 1.0784x; 1.0013x over previous
"""GNN node-update layer (segment-softmax message passing) on 8 TRN2 cores.

Math notes (validated vs reference in fp64: rel L2 ~1.5e-6):
  - scores_mean = h_dst @ mean(attn_W)[:128] + ef @ mean(attn_W)[128:] + mean(b).
    The h_dst term is constant within each dst segment, so it cancels in the
    segment softmax -> no node gather needed.  q_e = ef_e @ w_q.
  - q in [-0.9, 0.9] so exp(q) needs no max subtraction; the reference's
    max-shift only changes the EPS term by O(1e-5) relative.
  - softmax-weighted sum of edge values pushed through the linear projection:
      msg_n = (G_n @ ev_W)/(S_n+eps) + ev_b * S_n/(S_n+eps),
      G_n = sum_e exp(q_e) ef_e  (64-dim),  S_n = sum_e exp(q_e).
  - out = nf + relu(msg@W1+b1)@W2 + b2   (b2 folded into the nf stream).

Sharding: nodes partitioned into 8 contiguous ranges of 12500; each core gets
the edges whose dst lands in its range (no collectives).  Host sorts edges by
dst, groups them by 128-node tile, pads each group to 8 chunks of 128 edge
slots (max real count is ~900).  Pad slots carry dst sentinel 128 so their
one-hot column is all-zero and they contribute nothing.

Edge features ride in two fp16 halves (hi + residual lo) so the scatter
matmul runs at 1 cycle/row with ~fp32 accuracy: one 130-wide rhs
[ef_hi | 1 | ef_lo | 0] per chunk, hi/lo partial sums added in the epilogue.

Device per 128-edge chunk:
  q-mm   : lhsT = efT(65x128) fp16, rhs = wq(65x1)      -> q column (128x1)
  exp    : ACT on the 8 q columns of a group -> exp_all slice (128x8)
  one-hot: DVE ohx = (iota == dst) * exp  (two-op tensor_scalar, fp16 out)
  scatter: lhsT = ohx, rhs = combo(128x130) fp16, psum-accumulated over the
           group -> [G_hi | S | G_lo | 0] (128n x 130)
Group epilogue: g65 = hi+lo; r = 1/(S+eps); gh = g65*r; PE-transpose ->
  ghT(65x128).  Node batch (4 groups): msgT = evW65^T @ ghT;
  t1 = (W1^T msgT + b1) max 0 (DVE); t2 = W2^T t1; out = t2 + nfT -> DMA.

Hardware constraint honored throughout: matmul / tensor-scalar encodings fit
only ONE sync-wait, so every hot instruction is arranged to have exactly one
cross-engine dependency.  DMA-completion waits are absorbed by throwaway PE
load_weights / transpose reads ("absorb" ops); exp_all and the deep oh pool
avoid write-after-read waits entirely.
"""

import sys

sys.path.insert(0, "/opt/trn_rl_repo")

import numpy as np

import concourse.bass as bass
import concourse.bacc as bacc
import concourse.mybir as mybir
from concourse.tile import TileContext, add_dep_helper
from concourse.bass_utils import run_bass_kernel_spmd

P = 128
N_CORES = 8
N_NODES = 100000
NPC = 12500                # nodes per core
NTILE = 98                 # 128-node tiles per core
NODE_PAD = NTILE * P       # 12544
CPG = 8                    # chunks (of 128 edge slots) per node tile
NCH = NTILE * CPG          # 784 chunks per core
E_PAD = NCH * P            # 100352 edge slots per core
GB = 4                     # node tiles per MLP batch (N = 512)
EPS = 1e-6

F32 = mybir.dt.float32
F32R = mybir.dt.float32r
F16 = mybir.dt.float16


def _build_nc():
    nc = bacc.Bacc()
    eft_d = nc.dram_tensor("eft", (65, E_PAD), F16, kind="ExternalInput")
    efp_d = nc.dram_tensor("efp", (P, NCH * 130), F16, kind="ExternalInput")
    dst_d = nc.dram_tensor("dstc", (P, NCH), F32, kind="ExternalInput")
    nft_d = nc.dram_tensor("nft", (P, NODE_PAD), F32, kind="ExternalInput")
    wq_d = nc.dram_tensor("wq", (65, 1), F16, kind="ExternalInput")
    evw_d = nc.dram_tensor("evw", (65, P), F32R, kind="ExternalInput")
    w1_d = nc.dram_tensor("w1", (P, P), F32R, kind="ExternalInput")
    w2_d = nc.dram_tensor("w2", (P, P), F32R, kind="ExternalInput")
    b1_d = nc.dram_tensor("b1", (P, 1), F32, kind="ExternalInput")
    iota_d = nc.dram_tensor("iota", (P, P), F16, kind="ExternalInput")
    ident_d = nc.dram_tensor("ident", (P, P), F32, kind="ExternalInput")
    out_d = nc.dram_tensor("outt", (P, NODE_PAD), F32, kind="ExternalOutput")

    with TileContext(nc) as tc:
        with (
            tc.tile_pool(name="const", bufs=1) as cp,
            tc.tile_pool(name="sbuf", bufs=3) as sb,
            tc.tile_pool(name="quad", bufs=6) as sb4,
            tc.tile_pool(name="stream", bufs=16) as sb16,
            tc.tile_pool(name="ohp", bufs=12) as ohp,
            tc.tile_pool(name="pq", bufs=2, space="PSUM") as pq_pool,
            tc.tile_pool(name="pg", bufs=2, space="PSUM") as pg_pool,
            tc.tile_pool(name="pt", bufs=1, space="PSUM") as pt_pool,
            tc.tile_pool(name="pbig", bufs=3, space="PSUM") as pbig_pool,
        ):
            def dma(out, in_):
                return nc.sync.dma_start(out=out, in_=in_)

            wq_t = cp.tile([65, 1], F16)
            dma(out=wq_t[:], in_=wq_d[:])
            evw_t = cp.tile([65, P], F32R)
            dma(out=evw_t[:], in_=evw_d[:])
            w1_t = cp.tile([P, P], F32R)
            dma(out=w1_t[:], in_=w1_d[:])
            w2_t = cp.tile([P, P], F32R)
            dma(out=w2_t[:], in_=w2_d[:])
            b1_t = cp.tile([P, 1], F32)
            dma(out=b1_t[:], in_=b1_d[:])
            iota_t = cp.tile([P, P], F16)
            dma(out=iota_t[:], in_=iota_d[:])
            ident_t = cp.tile([P, P], F32)
            dma(out=ident_t[:], in_=ident_d[:])
            dst_t = cp.tile([P, NCH], F32)
            dma(out=dst_t[:], in_=dst_d[:])
            # exp_all: one long-lived tile -> ACT never sees a WAR wait.
            exp_all = cp.tile([P, NCH], F32)
            # fresh-column scratch: each col written exactly once, so the
            # writing instruction carries only its single genuine wait.
            sdend = cp.tile([P, NTILE], F32)    # S + eps
            recd = cp.tile([P, NTILE], F32)     # 1/(S+eps)
            ob_big = cp.tile([P, NODE_PAD], F32)  # full output staging

            batches = [
                list(range(s, min(s + GB, NTILE))) for s in range(0, NTILE, GB)
            ]
            for groups in batches:
                gbw = len(groups) * P
                g0 = groups[0]
                ecols = len(groups) * CPG * P
                ccols = len(groups) * CPG * 130
                eft_b = sb4.tile([65, GB * CPG * P], F16, tag="eftb")
                nc.scalar.dma_start(out=eft_b[:, :ecols],
                    in_=eft_d[:, g0 * CPG * P : g0 * CPG * P + ecols])
                combo_b = sb4.tile([P, GB * CPG * 130], F16, tag="combob")
                dma(out=combo_b[:, :ccols],
                    in_=efp_d[:, g0 * CPG * 130 : g0 * CPG * 130 + ccols])
                ghT_b = sb.tile([65, GB * P], F32R, tag="ghT")
                nb = len(groups) * CPG
                pq = pq_pool.tile([P, GB * CPG], F32, tag="q")
                for kk in range(nb):
                    nc.tensor.matmul(
                        out=pq[:, kk : kk + 1],
                        lhsT=eft_b[:, kk * P : (kk + 1) * P],
                        rhs=wq_t[:],
                        start=True,
                        stop=True,
                    )
                nc.scalar.activation(
                    out=exp_all[:, g0 * CPG : g0 * CPG + nb],
                    in_=pq[:, :nb],
                    func=mybir.ActivationFunctionType.Exp,
                )
                for j, g in enumerate(groups):
                    eft_sl = eft_b[:, j * CPG * P : (j + 1) * CPG * P]
                    combo = combo_b[:, j * CPG * 130 : (j + 1) * CPG * 130]
                    pg = pg_pool.tile([P, 130], F32, tag="G")
                    for k in range(CPG):
                        c = g * CPG + k
                        oh = ohp.tile([P, P], F16, tag="oh")
                        eng = nc.vector if (k % 8 < 3) else nc.gpsimd
                        eng.tensor_scalar(
                            oh[:],
                            iota_t[:],
                            dst_t[:, c : c + 1],
                            exp_all[:, c : c + 1],
                            mybir.AluOpType.is_equal,
                            mybir.AluOpType.mult,
                        )
                        nc.tensor.matmul(
                            out=pg[:],
                            lhsT=oh[:],
                            rhs=combo[:, k * 130 : (k + 1) * 130],
                            start=(k == 0),
                            stop=(k == CPG - 1),
                        )
                    # epilogue: r = 1/(S+eps); gh_{hi,lo} = G_{hi,lo}*r; the
                    # PE transpose pair adds hi+lo via PSUM accumulation.
                    nc.vector.tensor_scalar_add(
                        sdend[:, g : g + 1], pg[:, 64:65], EPS)
                    nc.vector.reciprocal(recd[:, g : g + 1], sdend[:, g : g + 1])
                    gh_full = sb.tile([P, 130], F32, tag="gh")
                    nc.vector.tensor_scalar_mul(gh_full[:], pg[:],
                                                recd[:, g : g + 1])
                    gh_hi = gh_full[:, 0:65]
                    gh_lo = gh_full[:, 65:130]
                    ptr = pt_pool.tile([P, P], F32, tag="T")
                    nc.tensor.matmul(
                        out=ptr[0:65, :], lhsT=gh_hi, rhs=ident_t[:],
                        is_transpose=True, start=True, stop=False,
                    )
                    nc.tensor.matmul(
                        out=ptr[0:65, :], lhsT=gh_lo, rhs=ident_t[:],
                        is_transpose=True, start=False, stop=True,
                    )
                    nc.vector.tensor_copy(
                        out=ghT_b[:, j * P : (j + 1) * P], in_=ptr[0:65, :]
                    )
                # node-side MLP batch
                col0 = groups[0] * P
                pmsg = pbig_pool.tile([P, GB * P], F32, tag="big")
                nc.tensor.matmul(
                    out=pmsg[:, :gbw],
                    lhsT=evw_t[:],
                    rhs=ghT_b[:, :gbw],
                    start=True,
                    stop=True,
                )
                msgT = sb.tile([P, GB * P], F32R, tag="msgT")
                nc.scalar.copy(out=msgT[:, :gbw], in_=pmsg[:, :gbw])
                pt1 = pbig_pool.tile([P, GB * P], F32, tag="big")
                nc.tensor.matmul(
                    out=pt1[:, :gbw],
                    lhsT=w1_t[:],
                    rhs=msgT[:, :gbw],
                    start=True,
                    stop=True,
                )
                t1 = sb.tile([P, GB * P], F32R, tag="t1")
                nc.vector.tensor_scalar(
                    t1[:, :gbw],
                    pt1[:, :gbw],
                    b1_t[:, 0:1],
                    0.0,
                    mybir.AluOpType.add,
                    mybir.AluOpType.max,
                )
                nf_sl = sb4.tile([P, GB * P], F32, tag="nf")
                nc.gpsimd.dma_start(
                    out=nf_sl[:, :gbw], in_=nft_d[:, col0 : col0 + gbw])
                pt2 = pbig_pool.tile([P, GB * P], F32, tag="big")
                nc.tensor.matmul(
                    out=pt2[:, :gbw],
                    lhsT=w2_t[:],
                    rhs=t1[:, :gbw],
                    start=True,
                    stop=False,
                )
                # residual add on PE: identity-matmul accumulates nfT into
                # the same PSUM tile; its only wait is the nf DMA.
                nc.tensor.matmul(
                    out=pt2[:, :gbw],
                    lhsT=ident_t[:],
                    rhs=nf_sl[:, :gbw],
                    start=False,
                    stop=True,
                )
                nc.scalar.copy(
                    out=ob_big[:, col0 : col0 + gbw], in_=pt2[:, :gbw])
            # single output DMA: one DVE wait, no slot churn; off-chain
            # (it is last, lane phase no longer matters)
            nc.gpsimd.dma_start(out=out_d[:], in_=ob_big[:])
    nc.finalize()
    return nc


def _prep(inputs):
    nf = np.asarray(inputs["node_features"], np.float32)
    ef = np.asarray(inputs["edge_features"], np.float32)
    dst = np.asarray(inputs["edge_index"])[1].astype(np.int64)
    attn_w = np.asarray(inputs["attn_W"], np.float64).mean(axis=1)
    w_q = attn_w[128:].astype(np.float32)

    order = np.argsort(dst, kind="stable")
    ef_s = ef[order]
    dst_s = dst[order]
    bounds = np.searchsorted(dst_s, np.arange(0, N_NODES + 1, NPC))

    evw65 = np.concatenate(
        [np.asarray(inputs["ev_W"], np.float32),
         np.asarray(inputs["ev_b"], np.float32)[None, :]], axis=0)
    wq65 = np.concatenate([w_q, np.zeros(1, np.float32)])[:, None]
    b2 = np.asarray(inputs["out2_b"], np.float32)

    common = {
        "wq": wq65.astype(np.float16),
        "evw": np.ascontiguousarray(evw65),
        "w1": np.ascontiguousarray(np.asarray(inputs["out1_W"], np.float32)),
        "w2": np.ascontiguousarray(np.asarray(inputs["out2_W"], np.float32)),
        "b1": np.ascontiguousarray(
            np.asarray(inputs["out1_b"], np.float32)[:, None]),
        "iota": np.tile(np.arange(P, dtype=np.float16), (P, 1)),
        "ident": np.eye(P, dtype=np.float32),
    }

    in_maps = []
    for core in range(N_CORES):
        lo, hi = bounds[core], bounds[core + 1]
        efk = ef_s[lo:hi]
        dstk = dst_s[lo:hi] - core * NPC
        tile_id = dstk >> 7
        counts = np.bincount(tile_id, minlength=NTILE)
        assert counts.max() <= CPG * P, f"tile overflow: {counts.max()}"
        cum = np.zeros(NTILE, np.int64)
        np.cumsum(counts[:-1], out=cum[1:])
        slots = tile_id * (CPG * P) + (np.arange(len(dstk)) - cum[tile_id])

        ef_pad = np.zeros((E_PAD, 65), np.float32)
        ef_pad[slots, :64] = efk
        ef_pad[:, 64] = 1.0
        hi16 = ef_pad.astype(np.float16)
        lo16 = (ef_pad - hi16.astype(np.float32)).astype(np.float16)
        block = np.concatenate([hi16, lo16], axis=1)  # (E_PAD, 130)
        efp = np.ascontiguousarray(
            block.reshape(NCH, P, 130).transpose(1, 0, 2).reshape(P, NCH * 130))

        dl = np.full(E_PAD, P, np.float32)
        dl[slots] = (dstk & 127).astype(np.float32)
        dstc = np.ascontiguousarray(dl.reshape(NCH, P).T)

        eft = np.empty((65, E_PAD), np.float16)
        eft[:64] = ef_pad[:, :64].T
        eft[64] = 1.0

        nfk = np.zeros((NODE_PAD, P), np.float32)
        nfk[:NPC] = nf[core * NPC : (core + 1) * NPC]
        nft = np.ascontiguousarray(nfk.T + b2[:, None])

        in_maps.append(
            dict(common, eft=np.ascontiguousarray(eft), efp=efp,
                 dstc=dstc, nft=nft))
    return in_maps


_CACHED_NC = None


def kernel(**inputs) -> np.ndarray:
    global _CACHED_NC
    in_maps = _prep(inputs)
    if _CACHED_NC is None:
        _CACHED_NC = _build_nc()
    res = run_bass_kernel_spmd(_CACHED_NC, in_maps, core_ids=list(range(N_CORES)))
    return assemble(res.results)


def assemble(results):
    out = np.empty((N_NODES, P), np.float32)
    for core in range(N_CORES):
        out[core * NPC : (core + 1) * NPC] = results[core]["outt"][:, :NPC].T
    return out


if __name__ == "__main__":
    rng = np.random.default_rng(0)
    demo = {
        "node_features": rng.standard_normal((N_NODES, P)).astype(np.float32),
        "edge_features": rng.standard_normal((640000, 64)).astype(np.float32),
        "attn_W": (rng.standard_normal((192, 4)) * 0.07).astype(np.float32),
        "attn_b": (rng.standard_normal(4) * 0.07).astype(np.float32),
        "ev_W": (rng.standard_normal((64, P)) * 0.125).astype(np.float32),
        "ev_b": (rng.standard_normal(P) * 0.125).astype(np.float32),
        "out1_W": (rng.standard_normal((P, P)) * 0.09).astype(np.float32),
        "out1_b": (rng.standard_normal(P) * 0.09).astype(np.float32),
        "out2_W": (rng.standard_normal((P, P)) * 0.09).astype(np.float32),
        "out2_b": (rng.standard_normal(P) * 0.09).astype(np.float32),
        "edge_index": rng.integers(0, N_NODES, (2, 640000)).astype(np.int32),
    }
    out = kernel(**demo)
    print("kernel ran:", out.shape, out.dtype, np.abs(out).max())
